# revision 25
# baseline (speedup 1.0000x reference)
"""v4: compact per-range pair-table gather (2 edges per DMA descriptor).

Per (core, 128-dst-node range): edges sorted by src; the range's unique
sorted srcs U define a compact rank space. A pair table holds rows
[x[U[j]] | x[U[j+1]]] (512B each), so a descriptor whose idx is j delivers
features for one edge with src U[j] (even lane, row bytes 0:128) and one
with src U[j+1] (odd lane, bytes 256:384). Since consecutive src-sorted
edges always have rank gap 0 or 1, a greedy chain packing fits ~2 edges per
descriptor: SWDGE descriptor count halves and 512B transfers avoid the
<512B DMA bus penalty. Aggregation is one-hot matmuls per lane into a
shared PSUM accumulator; host-computed 1/max(cnt,1) scales at eviction.
"""

import sys

if "/opt/trn_rl_repo" not in sys.path:
    sys.path.insert(0, "/opt/trn_rl_repo")

import numpy as np
import ml_dtypes

import concourse.tile as tile
from concourse import bacc, bass, mybir

P = 128
F = 64
TW = 128  # bf16 elems per node row (256B); pair row = 2*TW
N_NODES = 50000
N_CORES = 8
NPC = N_NODES // N_CORES
H = 64  # dst-range width
NR = (NPC + H - 1) // H  # 98 ranges of 64 dst nodes
R_LAST = NPC - (NR - 1) * H


def build_nc(
    b2: list,  # per-range slot blocks (128 slots each)
    urows: list,  # per-range pair-table rows (max over cores, padded)
    tot_rows: int,
    onehot_batch: int = 8,
    msg_bufs: int = 14,
    psum_bufs: int = 8,
    oh_bufs: int = 5,
    n_queues: int = 4,
):
    dt_x = mybir.dt.bfloat16
    nc = bacc.Bacc(num_swdge_queues=n_queues)
    b2max = max(b2)
    # meta layout per range: [idx (8*b2) | dst_even (b2) | dst_odd (b2)]
    offs = []
    o = 0
    for b in b2:
        offs.append(o)
        o += 10 * b
    w_total = o

    ptab_ext = nc.declare_dram_parameter("ptab", [tot_rows, 2 * TW], dt_x, isOutput=False)
    meta_ext = nc.declare_dram_parameter("meta16", [P, w_total], mybir.dt.int16, isOutput=False)
    recip_ext = nc.declare_dram_parameter("recip", [P, NR], mybir.dt.float32, isOutput=False)
    out_ext = nc.declare_dram_parameter("out", [NPC, F], mybir.dt.float32, isOutput=True)

    qn = 0
    with tile.TileContext(nc) as tc:
        with (
            tc.tile_pool(name="const", bufs=1) as const_pool,
            tc.tile_pool(name="msg", bufs=msg_bufs) as msg_pool,
            tc.tile_pool(name="onehot", bufs=oh_bufs) as oh_pool,
            tc.tile_pool(name="evict", bufs=2) as ev_pool,
            tc.tile_pool(name="psum", bufs=psum_bufs, space="PSUM") as psum_pool,
        ):
            iota_i = const_pool.tile([P, 1, P], mybir.dt.int32)
            nc.gpsimd.iota(iota_i[:], pattern=[[1, P]], base=0, channel_multiplier=0)
            iota_c = const_pool.tile([P, 1, P], dt_x)
            nc.vector.tensor_copy(out=iota_c[:], in_=iota_i[:])

            meta_t = const_pool.tile([P, w_total], mybir.dt.int16)
            nc.sync.dma_start(out=meta_t[:], in_=meta_ext[:, :])
            recip_t = const_pool.tile([P, NR], mybir.dt.float32)
            nc.sync.dma_start(out=recip_t[:], in_=recip_ext[:, :])

            row0 = 0
            for r in range(NR):
                rows = H if r < NR - 1 else R_LAST
                b = b2[r]
                o0 = offs[r]
                msg_t = msg_pool.tile([P, b2max, 2 * TW], dt_x)
                nc.gpsimd.dma_gather(
                    out_ap=msg_t[:, :b, :],
                    in_ap=ptab_ext[row0 : row0 + urows[r], :],
                    idxs_ap=meta_t[:, o0 : o0 + 8 * b],
                    num_idxs=P * b,
                    num_idxs_reg=P * b,
                    elem_size=2 * TW,
                    queue_num=qn % n_queues,
                    single_packet=False,
                )
                qn += 1
                row0 += urows[r]

                psum_t = psum_pool.tile([H, F], mybir.dt.float32)
                # both lanes' dst values are contiguous in meta: one is_equal
                # builds the even one-hots (cols 0:b) and odd (cols b:2b).
                dst_eo = meta_t[:, o0 + 8 * b : o0 + 10 * b].bitcast(dt_x)
                oh_t = oh_pool.tile([P, 2 * b2max, H], dt_x)
                nc.vector.tensor_tensor(
                    out=oh_t[:, : 2 * b, :],
                    in0=dst_eo[:, :, None].to_broadcast([P, 2 * b, H]),
                    in1=iota_c[:, :, 0:H].to_broadcast([P, 2 * b, H]),
                    op=mybir.AluOpType.is_equal,
                )
                for j in range(b):
                    nc.tensor.matmul(
                        out=psum_t[:],
                        lhsT=oh_t[:, j, :],
                        rhs=msg_t[:, j, 0:F],
                        start=(j == 0),
                        stop=False,
                    )
                    nc.tensor.matmul(
                        out=psum_t[:],
                        lhsT=oh_t[:, b + j, :],
                        rhs=msg_t[:, j, TW : TW + F],
                        start=False,
                        stop=(j == b - 1),
                    )

                out_t = ev_pool.tile([H, F], mybir.dt.float32)
                nc.scalar.activation(
                    out_t[:],
                    psum_t[:],
                    func=mybir.ActivationFunctionType.Copy,
                    scale=recip_t[0:H, r : r + 1],
                )
                nc.sync.dma_start(out=out_ext[r * H : r * H + rows], in_=out_t[:rows])
    nc.compile()
    return nc


def _pack_idx(idx: np.ndarray, n_blocks: int) -> np.ndarray:
    w = 8 * n_blocks
    out16 = np.zeros((16, w), dtype=np.int16)
    if len(idx):
        i = np.arange(len(idx))
        out16[i % 16, i // 16] = idx.astype(np.int16)
    return np.tile(out16, (8, 1))


def _pack_slots(vals: np.ndarray, n_blocks: int, fill: float) -> np.ndarray:
    """Slot i -> [partition i%128, block i//128], bf16 viewed as int16."""
    out = np.full((P, n_blocks), fill, dtype=np.float32)
    if len(vals):
        i = np.arange(len(vals))
        out[i % P, i // P] = vals
    return out.astype(ml_dtypes.bfloat16).view(np.int16)


def _chain_pack(rank: np.ndarray, dl: np.ndarray, u: int):
    """Greedy chain packing: desc idx k serves one even-lane edge (src U[k])
    and one odd-lane edge (src U[k+1]). Returns (desc_idx, dst_even, dst_odd).
    rank/dl are src-sorted."""
    m = np.bincount(rank, minlength=u) if u else np.zeros(0, np.int64)
    # edges grouped by rank, in order
    desc_idx = []
    dst_e = []
    dst_o = []
    pend = []  # desc positions whose odd lane accepts current k
    pos = 0
    for k in range(u):
        cnt = m[k]
        vals = dl[pos : pos + cnt]
        pos += cnt
        take = min(len(pend), cnt)
        for t in range(take):
            dst_o[pend[t]] = vals[t]
        new_pend = []
        for v in vals[take:]:
            desc_idx.append(k)
            dst_e.append(v)
            dst_o.append(-1.0)
            new_pend.append(len(desc_idx) - 1)
        pend = new_pend
    return (
        np.asarray(desc_idx, dtype=np.int64),
        np.asarray(dst_e, dtype=np.float32),
        np.asarray(dst_o, dtype=np.float32),
    )


def shard_inputs(x: np.ndarray, edge_idx: np.ndarray):
    src = np.ascontiguousarray(edge_idx[0]).astype(np.int64)
    dst = np.ascontiguousarray(edge_idx[1]).astype(np.int64)

    order = np.argsort(dst, kind="stable")
    src_s = src[order]
    dst_s = dst[order]

    cnt = np.bincount(dst, minlength=N_NODES)
    recip = (1.0 / np.maximum(cnt, 1)).astype(np.float32)

    xx = np.zeros((N_NODES, TW), dtype=ml_dtypes.bfloat16)
    xx[:, :F] = x.astype(ml_dtypes.bfloat16)

    core_bounds = np.searchsorted(dst_s, np.arange(N_CORES + 1) * NPC)

    # first pass: per (core, range) packing
    packed = [[None] * NR for _ in range(N_CORES)]
    uniq = [[None] * NR for _ in range(N_CORES)]
    for c in range(N_CORES):
        s0, s1 = core_bounds[c], core_bounds[c + 1]
        cs_all = src_s[s0:s1]
        cd_all = dst_s[s0:s1] - c * NPC
        chunk_bounds = np.searchsorted(cd_all, np.arange(NR + 1) * H)
        for r in range(NR):
            a, bnd = chunk_bounds[r], chunk_bounds[r + 1]
            sl = cs_all[a:bnd]
            dl = (cd_all[a:bnd] - r * H).astype(np.float32)
            so = np.argsort(sl, kind="stable")
            ss = sl[so]
            dd = dl[so]
            U, rank = np.unique(ss, return_inverse=True)
            di, de, do = _chain_pack(rank, dd, len(U))
            packed[c][r] = (di, de, do)
            uniq[c][r] = U

    b2 = []
    urows = []
    for r in range(NR):
        smax = max(len(packed[c][r][0]) for c in range(N_CORES))
        b2.append(max(1, (smax + P - 1) // P))
        urows.append(max(2, max(len(uniq[c][r]) for c in range(N_CORES))))
    tot_rows = sum(urows)

    offs = []
    o = 0
    for b in b2:
        offs.append(o)
        o += 10 * b
    w_total = o

    in_maps = []
    for c in range(N_CORES):
        meta16 = np.zeros((P, w_total), dtype=np.int16)
        ptab = np.zeros((tot_rows, 2 * TW), dtype=ml_dtypes.bfloat16)
        row0 = 0
        for r in range(NR):
            di, de, do = packed[c][r]
            U = uniq[c][r]
            b = b2[r]
            o0 = offs[r]
            pad_i = np.zeros(b * P, dtype=np.int64)
            pad_i[: len(di)] = di
            meta16[:, o0 : o0 + 8 * b] = _pack_idx(pad_i, b)
            meta16[:, o0 + 8 * b : o0 + 9 * b] = _pack_slots(de, b, -1.0)
            meta16[:, o0 + 9 * b : o0 + 10 * b] = _pack_slots(do, b, -1.0)
            u = len(U)
            if u:
                ptab[row0 : row0 + u, :TW] = xx[U]
                nxt = np.minimum(np.arange(1, u + 1), u - 1)
                ptab[row0 : row0 + u, TW:] = xx[U[nxt]]
            row0 += urows[r]
        rfull = np.zeros(NR * H, dtype=np.float32)
        rfull[:NPC] = recip[c * NPC : (c + 1) * NPC]
        rmat = np.zeros((P, NR), dtype=np.float32)
        rmat[:H] = rfull.reshape(NR, H).T
        in_maps.append({"ptab": ptab, "meta16": meta16, "recip": rmat})

    return in_maps, b2, urows, tot_rows


def run(x, edge_idx, trace: bool = False):
    from concourse.bass_utils import run_bass_kernel_spmd

    x = np.asarray(x)
    edge_idx = np.asarray(edge_idx)
    in_maps, b2, urows, tot_rows = shard_inputs(x, edge_idx)
    nc = build_nc(b2, urows, tot_rows)
    res = run_bass_kernel_spmd(nc, in_maps, core_ids=list(range(N_CORES)), trace=trace)
    out = np.concatenate([r["out"] for r in res.results], axis=0)
    return out.astype(np.float32), res.exec_time_ns


def kernel(x, edge_idx):
    out, _ = run(x, edge_idx)
    return out


# revision 27
# speedup vs baseline: 1.0812x; 1.0812x over previous
"""v4: compact per-range pair-table gather (2 edges per DMA descriptor).

Per (core, 128-dst-node range): edges sorted by src; the range's unique
sorted srcs U define a compact rank space. A pair table holds rows
[x[U[j]] | x[U[j+1]]] (512B each), so a descriptor whose idx is j delivers
features for one edge with src U[j] (even lane, row bytes 0:128) and one
with src U[j+1] (odd lane, bytes 256:384). Since consecutive src-sorted
edges always have rank gap 0 or 1, a greedy chain packing fits ~2 edges per
descriptor: SWDGE descriptor count halves and 512B transfers avoid the
<512B DMA bus penalty. Aggregation is one-hot matmuls per lane into a
shared PSUM accumulator; host-computed 1/max(cnt,1) scales at eviction.
"""

import sys

if "/opt/trn_rl_repo" not in sys.path:
    sys.path.insert(0, "/opt/trn_rl_repo")

import numpy as np
import ml_dtypes

import concourse.tile as tile
from concourse import bacc, bass, mybir

P = 128
F = 64
TW = 128  # bf16 elems per node row (256B); pair row = 2*TW
N_NODES = 50000
N_CORES = 8
NPC = N_NODES // N_CORES
NR = (NPC + P - 1) // P
R_LAST = NPC - (NR - 1) * P


def build_nc(
    b2: list,  # per-range slot blocks (128 slots each)
    urows: list,  # per-range pair-table rows (max over cores, padded)
    tot_rows: int,
    onehot_batch: int = 8,
    msg_bufs: int = 14,
    psum_bufs: int = 8,
    oh_bufs: int = 5,
    n_queues: int = 4,
):
    dt_x = mybir.dt.bfloat16
    nc = bacc.Bacc(num_swdge_queues=n_queues)
    b2max = max(b2)
    # meta layout per range: [idx (8*b2) | dst_even (b2) | dst_odd (b2)]
    offs = []
    o = 0
    for b in b2:
        offs.append(o)
        o += 10 * b
    w_total = o

    ptab_ext = nc.declare_dram_parameter("ptab", [tot_rows, 2 * TW], dt_x, isOutput=False)
    meta_ext = nc.declare_dram_parameter("meta16", [P, w_total], mybir.dt.int16, isOutput=False)
    recip_ext = nc.declare_dram_parameter("recip", [P, NR], mybir.dt.float32, isOutput=False)
    out_ext = nc.declare_dram_parameter("out", [NPC, F], mybir.dt.float32, isOutput=True)

    qn = 0
    with tile.TileContext(nc) as tc:
        with (
            tc.tile_pool(name="const", bufs=1) as const_pool,
            tc.tile_pool(name="msg", bufs=msg_bufs) as msg_pool,
            tc.tile_pool(name="onehot", bufs=oh_bufs) as oh_pool,
            tc.tile_pool(name="evict", bufs=2) as ev_pool,
            tc.tile_pool(name="psum", bufs=psum_bufs, space="PSUM") as psum_pool,
        ):
            iota_i = const_pool.tile([P, 1, P], mybir.dt.int32)
            nc.gpsimd.iota(iota_i[:], pattern=[[1, P]], base=0, channel_multiplier=0)
            iota_c = const_pool.tile([P, 1, P], dt_x)
            nc.vector.tensor_copy(out=iota_c[:], in_=iota_i[:])

            meta_t = const_pool.tile([P, w_total], mybir.dt.int16)
            nc.sync.dma_start(out=meta_t[:], in_=meta_ext[:, :])
            recip_t = const_pool.tile([P, NR], mybir.dt.float32)
            nc.sync.dma_start(out=recip_t[:], in_=recip_ext[:, :])

            row0 = 0
            for r in range(NR):
                rows = P if r < NR - 1 else R_LAST
                b = b2[r]
                o0 = offs[r]
                msg_t = msg_pool.tile([P, b2max, 2 * TW], dt_x)
                nc.gpsimd.dma_gather(
                    out_ap=msg_t[:, :b, :],
                    in_ap=ptab_ext[row0 : row0 + urows[r], :],
                    idxs_ap=meta_t[:, o0 : o0 + 8 * b],
                    num_idxs=P * b,
                    num_idxs_reg=P * b,
                    elem_size=2 * TW,
                    queue_num=qn % n_queues,
                    single_packet=False,
                )
                qn += 1
                row0 += urows[r]

                psum_t = psum_pool.tile([P, F], mybir.dt.float32)
                # both lanes' dst values are contiguous in meta: one is_equal
                # builds the even one-hots (cols 0:b) and odd (cols b:2b).
                dst_eo = meta_t[:, o0 + 8 * b : o0 + 10 * b].bitcast(dt_x)
                oh_t = oh_pool.tile([P, 2 * b2max, P], dt_x)
                nc.vector.tensor_tensor(
                    out=oh_t[:, : 2 * b, :],
                    in0=dst_eo[:, :, None].to_broadcast([P, 2 * b, P]),
                    in1=iota_c[:].to_broadcast([P, 2 * b, P]),
                    op=mybir.AluOpType.is_equal,
                )
                for j in range(b):
                    nc.tensor.matmul(
                        out=psum_t[:],
                        lhsT=oh_t[:, j, :],
                        rhs=msg_t[:, j, 0:F],
                        start=(j == 0),
                        stop=False,
                    )
                for j in range(b):
                    nc.tensor.matmul(
                        out=psum_t[:],
                        lhsT=oh_t[:, b + j, :],
                        rhs=msg_t[:, j, TW : TW + F],
                        start=False,
                        stop=(j == b - 1),
                    )

                out_t = ev_pool.tile([P, F], mybir.dt.float32)
                nc.scalar.activation(
                    out_t[:],
                    psum_t[:],
                    func=mybir.ActivationFunctionType.Copy,
                    scale=recip_t[:, r : r + 1],
                )
                nc.sync.dma_start(out=out_ext[r * P : r * P + rows], in_=out_t[:rows])
    nc.compile()
    return nc


def _pack_idx(idx: np.ndarray, n_blocks: int) -> np.ndarray:
    w = 8 * n_blocks
    out16 = np.zeros((16, w), dtype=np.int16)
    if len(idx):
        i = np.arange(len(idx))
        out16[i % 16, i // 16] = idx.astype(np.int16)
    return np.tile(out16, (8, 1))


def _pack_slots(vals: np.ndarray, n_blocks: int, fill: float) -> np.ndarray:
    """Slot i -> [partition i%128, block i//128], bf16 viewed as int16."""
    out = np.full((P, n_blocks), fill, dtype=np.float32)
    if len(vals):
        i = np.arange(len(vals))
        out[i % P, i // P] = vals
    return out.astype(ml_dtypes.bfloat16).view(np.int16)


def _chain_pack(rank: np.ndarray, dl: np.ndarray, u: int):
    """Greedy chain packing: desc idx k serves one even-lane edge (src U[k])
    and one odd-lane edge (src U[k+1]). Returns (desc_idx, dst_even, dst_odd).
    rank/dl are src-sorted."""
    m = np.bincount(rank, minlength=u) if u else np.zeros(0, np.int64)
    # edges grouped by rank, in order
    desc_idx = []
    dst_e = []
    dst_o = []
    pend = []  # desc positions whose odd lane accepts current k
    pos = 0
    for k in range(u):
        cnt = m[k]
        vals = dl[pos : pos + cnt]
        pos += cnt
        take = min(len(pend), cnt)
        for t in range(take):
            dst_o[pend[t]] = vals[t]
        new_pend = []
        for v in vals[take:]:
            desc_idx.append(k)
            dst_e.append(v)
            dst_o.append(-1.0)
            new_pend.append(len(desc_idx) - 1)
        pend = new_pend
    return (
        np.asarray(desc_idx, dtype=np.int64),
        np.asarray(dst_e, dtype=np.float32),
        np.asarray(dst_o, dtype=np.float32),
    )


def shard_inputs(x: np.ndarray, edge_idx: np.ndarray):
    src = np.ascontiguousarray(edge_idx[0]).astype(np.int64)
    dst = np.ascontiguousarray(edge_idx[1]).astype(np.int64)

    order = np.argsort(dst, kind="stable")
    src_s = src[order]
    dst_s = dst[order]

    cnt = np.bincount(dst, minlength=N_NODES)
    recip = (1.0 / np.maximum(cnt, 1)).astype(np.float32)

    xx = np.zeros((N_NODES, TW), dtype=ml_dtypes.bfloat16)
    xx[:, :F] = x.astype(ml_dtypes.bfloat16)

    core_bounds = np.searchsorted(dst_s, np.arange(N_CORES + 1) * NPC)

    # first pass: per (core, range) packing
    packed = [[None] * NR for _ in range(N_CORES)]
    uniq = [[None] * NR for _ in range(N_CORES)]
    for c in range(N_CORES):
        s0, s1 = core_bounds[c], core_bounds[c + 1]
        cs_all = src_s[s0:s1]
        cd_all = dst_s[s0:s1] - c * NPC
        chunk_bounds = np.searchsorted(cd_all, np.arange(NR + 1) * P)
        for r in range(NR):
            a, bnd = chunk_bounds[r], chunk_bounds[r + 1]
            sl = cs_all[a:bnd]
            dl = (cd_all[a:bnd] - r * P).astype(np.float32)
            so = np.argsort(sl, kind="stable")
            ss = sl[so]
            dd = dl[so]
            U, rank = np.unique(ss, return_inverse=True)
            di, de, do = _chain_pack(rank, dd, len(U))
            packed[c][r] = (di, de, do)
            uniq[c][r] = U

    b2 = []
    urows = []
    for r in range(NR):
        smax = max(len(packed[c][r][0]) for c in range(N_CORES))
        b2.append(max(1, (smax + P - 1) // P))
        urows.append(max(2, max(len(uniq[c][r]) for c in range(N_CORES))))
    tot_rows = sum(urows)

    offs = []
    o = 0
    for b in b2:
        offs.append(o)
        o += 10 * b
    w_total = o

    in_maps = []
    for c in range(N_CORES):
        meta16 = np.zeros((P, w_total), dtype=np.int16)
        ptab = np.zeros((tot_rows, 2 * TW), dtype=ml_dtypes.bfloat16)
        row0 = 0
        for r in range(NR):
            di, de, do = packed[c][r]
            U = uniq[c][r]
            b = b2[r]
            o0 = offs[r]
            pad_i = np.zeros(b * P, dtype=np.int64)
            pad_i[: len(di)] = di
            meta16[:, o0 : o0 + 8 * b] = _pack_idx(pad_i, b)
            meta16[:, o0 + 8 * b : o0 + 9 * b] = _pack_slots(de, b, -1.0)
            meta16[:, o0 + 9 * b : o0 + 10 * b] = _pack_slots(do, b, -1.0)
            u = len(U)
            if u:
                ptab[row0 : row0 + u, :TW] = xx[U]
                nxt = np.minimum(np.arange(1, u + 1), u - 1)
                ptab[row0 : row0 + u, TW:] = xx[U[nxt]]
            row0 += urows[r]
        rfull = np.zeros(NR * P, dtype=np.float32)
        rfull[:NPC] = recip[c * NPC : (c + 1) * NPC]
        rmat = rfull.reshape(NR, P).T.copy()
        in_maps.append({"ptab": ptab, "meta16": meta16, "recip": rmat})

    return in_maps, b2, urows, tot_rows


def run(x, edge_idx, trace: bool = False):
    from concourse.bass_utils import run_bass_kernel_spmd

    x = np.asarray(x)
    edge_idx = np.asarray(edge_idx)
    in_maps, b2, urows, tot_rows = shard_inputs(x, edge_idx)
    nc = build_nc(b2, urows, tot_rows)
    res = run_bass_kernel_spmd(nc, in_maps, core_ids=list(range(N_CORES)), trace=trace)
    out = np.concatenate([r["out"] for r in res.results], axis=0)
    return out.astype(np.float32), res.exec_time_ns


def kernel(x, edge_idx):
    out, _ = run(x, edge_idx)
    return out
